# revision 18
# baseline (speedup 1.0000x reference)
"""Bahdanau attention TRN2 Bass kernel.

Data-parallel over batch across 8 NeuronCores (8 batch rows per core).
Single pass over encoder_outputs per core (fused scores + softmax + context):

  per (batch b, pair of S-chunks of 128 rows):
    enc_sb [128(S), 2, 1024(E)] <- SWDGE cast-DMA fp32->bf16 (natural layout)
    per chunk: encT [E, S] via 8x PE transpose (bf16) -> PSUM -> SBUF copy
               enc_proj [S, A] = encT.T @ W_enc (8 K-steps bf16, PSUM accum)
    z = enc_proj + dec_proj[b] (DVE); tanh (ACT)
    scores[S,2] = reduce_add(tanh * v) + mask_bias (DVE)
    w = exp(scores) (ACT; masked rows carry -1e10 bias -> exp == 0)
    ctx[1,1024] += w.T @ enc_sb  (bf16, PSUM accumulate across chunks)
  batch end: denom via ones-matmul (fused broadcast to 32 partitions),
  normalize ctx and attention weights.

Scores are bounded (|score| <= sum|v| ~ 10), so no running-max is needed and
exp/softmax matches the reference (which subtracts the max) to fp32 rounding.
decoder_hidden and mask are fed in DMA-friendly layouts (host-side transpose
only; all arithmetic stays on device).
"""

import sys

sys.path.insert(0, "/opt/trn_rl_repo")

import numpy as np
from contextlib import ExitStack

import concourse.bass as bass
import concourse.tile as tile
from concourse import bacc, mybir
from concourse.bass import ds, ts
from concourse.bass_utils import run_bass_kernel_spmd
from concourse.masks import make_identity

F32 = mybir.dt.float32
BF16 = mybir.dt.bfloat16
U8 = mybir.dt.uint8

# problem shape (hardcoded per harness contract)
B, S, DEC, ENC, A = 64, 4096, 512, 1024, 256
NCORES = 8
BPC = B // NCORES  # batches per core
SC = 128           # S rows per chunk
NCH = S // SC      # 32 chunks

COPY_ACT_K = 4     # of 8 encT k-slices copied on ACT (rest on DVE)


def build_program(n_batches=BPC, n_chunks=NCH, num_devices=NCORES,
                  enc_bufs=4, copy_act_k=COPY_ACT_K,
                  pst_bufs=2, psep_bufs=2, psctx_bufs=2, enct_bufs=3,
                  work_bufs=3):
    nc = bacc.Bacc("TRN2", target_bir_lowering=False, debug=False,
                   num_devices=num_devices)

    dh_d = nc.dram_tensor("decoder_hidden_t", [DEC, BPC], F32, kind="ExternalInput")
    enc_d = nc.dram_tensor("encoder_outputs", [BPC, S, ENC], F32, kind="ExternalInput")
    mask_d = nc.dram_tensor("mask_t", [BPC, SC, NCH], U8, kind="ExternalInput")
    wdec_d = nc.dram_tensor("W_dec", [DEC, A], F32, kind="ExternalInput")
    wenc_d = nc.dram_tensor("W_enc", [ENC, A], F32, kind="ExternalInput")
    v_d = nc.dram_tensor("v", [A], F32, kind="ExternalInput")
    ctx_d = nc.dram_tensor("context_out", [BPC, ENC], F32, kind="ExternalOutput")
    attn_d = nc.dram_tensor("attn_out", [BPC, S], F32, kind="ExternalOutput")

    KE = ENC // 128  # 8 contraction chunks for enc proj
    KD = DEC // 128  # 4 contraction chunks for dec proj
    npairs = n_chunks // 2

    with ExitStack() as ctx:
        tc = ctx.enter_context(tile.TileContext(nc))
        singles = ctx.enter_context(tc.tile_pool(name="singles", bufs=1))
        encp = ctx.enter_context(tc.tile_pool(name="encp", bufs=enc_bufs))
        enctp = ctx.enter_context(tc.tile_pool(name="enctp", bufs=enct_bufs))
        work = ctx.enter_context(tc.tile_pool(name="work", bufs=work_bufs))
        perb = ctx.enter_context(tc.tile_pool(name="perb", bufs=2))
        outp = ctx.enter_context(tc.tile_pool(name="outp", bufs=2))
        ps_t = ctx.enter_context(tc.tile_pool(name="ps_t", bufs=pst_bufs, space="PSUM"))
        ps_ep = ctx.enter_context(tc.tile_pool(name="ps_ep", bufs=psep_bufs, space="PSUM"))
        ps_ctx = ctx.enter_context(tc.tile_pool(name="ps_ctx", bufs=psctx_bufs, space="PSUM"))
        dramp = ctx.enter_context(tc.tile_pool(name="dramp", bufs=1, space="DRAM"))

        # ---- constants / prologue ----
        ident = singles.tile([128, 128], F32)
        make_identity(nc, ident[:])
        ident_b = singles.tile([128, 128], BF16)
        nc.vector.tensor_copy(ident_b[:], ident[:])

        wenc_sb = singles.tile([128, KE, A], BF16)
        nc.gpsimd.dma_start(out=wenc_sb[:],
                            in_=wenc_d.ap().rearrange("(k p) a -> p k a", p=128))
        wdec_sb = singles.tile([128, KD, A], F32)
        nc.sync.dma_start(out=wdec_sb[:],
                          in_=wdec_d.ap().rearrange("(k p) a -> p k a", p=128))
        v_bc = singles.tile([128, 1, A], F32)
        nc.sync.dma_start(out=v_bc[:],
                          in_=v_d.ap()[None, None, :].to_broadcast((128, 1, A)))
        ones32 = singles.tile([128, NCH], F32)
        nc.vector.memset(ones32[:], 1.0)

        dhT = singles.tile([128, KD, BPC], F32)
        nc.sync.dma_start(out=dhT[:],
                          in_=dh_d.ap().rearrange("(k p) b -> p k b", p=128))
        # dec_proj [BPC, A]
        dp_ps = ps_ep.tile([128, 2, A], F32, tag="ps_ep")
        for k in range(KD):
            nc.tensor.matmul(dp_ps[:BPC, 0, :], dhT[:, k, :], wdec_sb[:, k, :],
                             start=(k == 0), stop=(k == KD - 1))
        dp_sb = singles.tile([BPC, A], F32)
        nc.vector.tensor_copy(dp_sb[:], dp_ps[:BPC, 0, :])
        dp_dram = dramp.tile([BPC, A], F32)
        nc.sync.dma_start(out=dp_dram[:], in_=dp_sb[:])
        dec_bc_all = singles.tile([128, BPC, A], F32)
        nc.sync.dma_start(out=dec_bc_all[:],
                          in_=dp_dram[None, :, :].to_broadcast((128, BPC, A)))

        mask_all = singles.tile([SC, BPC, NCH], U8)
        nc.sync.dma_start(out=mask_all[:],
                          in_=mask_d.ap().rearrange("b p c -> p b c"))
        mbias_all = singles.tile([128, BPC, NCH], F32)
        nc.scalar.activation(mbias_all[:], mask_all[:],
                             mybir.ActivationFunctionType.Copy,
                             bias=0.0, scale=-1e10)

        for b in range(n_batches):
            dec_bc = dec_bc_all[:, b:b + 1, :]
            mbiasT = mbias_all[:, b, :]

            w_all = perb.tile([128, NCH], F32)
            ctx_ps = [ps_ctx.tile([1, 512], F32, tag=f"ps_ctx{i}", name=f"ctx_ps{i}_{b}")
                      for i in range(2)]

            for q in range(npairs):
                enc_sb = encp.tile([128, 2, ENC], BF16)
                nc.gpsimd.dma_start(
                    out=enc_sb[:],
                    in_=enc_d[b, ds(q * 2 * SC, 2 * SC), :]
                        .rearrange("(j p) e -> p j e", p=SC))

                ep_ps = ps_ep.tile([128, 2, A], F32, tag="ps_ep")
                for j in range(2):
                    encT_ps = ps_t.tile([128, KE, 128], BF16)
                    for k in range(KE):
                        nc.tensor.transpose(encT_ps[:, k, :],
                                            enc_sb[:, j, ts(k, 128)], ident_b[:])
                    encT = enctp.tile([128, KE, 128], BF16)
                    ka = copy_act_k
                    nc.scalar.copy(encT[:, :ka, :], encT_ps[:, :ka, :])
                    nc.vector.tensor_copy(encT[:, ka:, :], encT_ps[:, ka:, :])
                    for k in range(KE):
                        nc.tensor.matmul(ep_ps[:, j, :], encT[:, k, :],
                                         wenc_sb[:, k, :],
                                         start=(k == 0), stop=(k == KE - 1))

                # z = enc_proj + dec_proj[b]; tanh; scores = v . tanh + mask
                z_sb = work.tile([128, 2, A], F32)
                nc.vector.tensor_add(z_sb[:], ep_ps[:],
                                     dec_bc.to_broadcast((128, 2, A)))
                th_sb = work.tile([128, 2, A], F32)
                nc.scalar.activation(th_sb[:], z_sb[:],
                                     mybir.ActivationFunctionType.Tanh)
                prod_sb = work.tile([128, 2, A], F32)
                nc.vector.tensor_mul(prod_sb[:], th_sb[:],
                                     v_bc[:].to_broadcast((128, 2, A)))
                sc_sb = work.tile([128, 2], F32)
                nc.vector.reduce_sum(sc_sb[:], prod_sb[:],
                                     axis=mybir.AxisListType.X)
                scm_sb = work.tile([128, 2], F32)
                nc.vector.tensor_add(scm_sb[:], sc_sb[:],
                                     mbiasT[:, 2 * q:2 * q + 2])
                # w = exp(scores); masked rows underflow to exactly 0
                nc.scalar.activation(w_all[:, 2 * q:2 * q + 2], scm_sb[:],
                                     mybir.ActivationFunctionType.Exp)
                w_bf = work.tile([128, 2], BF16)
                nc.vector.tensor_copy(w_bf[:], w_all[:, 2 * q:2 * q + 2])

                # ctx += w.T @ enc
                for j in range(2):
                    c = 2 * q + j
                    for i in range(2):
                        nc.tensor.matmul(ctx_ps[i][:], w_bf[:, j:j + 1],
                                         enc_sb[:, j, ts(i, 512)],
                                         start=(c == 0), stop=(c == n_chunks - 1),
                                         skip_group_check=True)

            # ---- batch epilogue ----
            dsum = outp.tile([128, 1], F32)
            nc.vector.reduce_sum(dsum[:], w_all[:, :n_chunks],
                                 axis=mybir.AxisListType.X)
            den_ps = ps_ep.tile([128, 2, A], F32, tag="ps_ep")
            nc.tensor.matmul(den_ps[:NCH, 0, :1], ones32[:], dsum[:],
                             start=True, stop=True)
            den32 = outp.tile([NCH, 1], F32)
            nc.vector.tensor_copy(den32[:], den_ps[:NCH, 0, :1])
            rden32 = outp.tile([NCH, 1], F32)
            nc.vector.reciprocal(rden32[:], den32[:])

            # context out
            ctx_sb = outp.tile([1, ENC], F32)
            for i in range(2):
                nc.vector.tensor_scalar_mul(ctx_sb[:, ts(i, 512)], ctx_ps[i][:],
                                            rden32[:1, :])
            nc.sync.dma_start(out=ctx_d[b][None, :], in_=ctx_sb[:])

            # attention weights out: transpose w_all -> [chunk, S-in-chunk]
            wT_ps = ps_ep.tile([128, 2, A], F32, tag="ps_ep")
            nc.tensor.transpose(wT_ps[:n_chunks, 0, :SC], w_all[:, :n_chunks],
                                ident[:])
            wout = outp.tile([NCH, SC], F32)
            nc.vector.tensor_scalar_mul(wout[:n_chunks], wT_ps[:n_chunks, 0, :SC],
                                        rden32[:n_chunks])
            nc.sync.dma_start(
                out=attn_d[b].rearrange("(c p) -> c p", p=SC)[:n_chunks],
                in_=wout[:n_chunks])

    nc.compile()
    return nc


_NC = None


def _get_nc():
    global _NC
    if _NC is None:
        _NC = build_program()
    return _NC


def kernel(decoder_hidden, encoder_outputs, mask, W_dec, W_enc, v,
           trace=False, **run_kwargs):
    decoder_hidden = np.ascontiguousarray(np.asarray(decoder_hidden, dtype=np.float32))
    encoder_outputs = np.ascontiguousarray(np.asarray(encoder_outputs, dtype=np.float32))
    mask_u8 = np.ascontiguousarray(np.asarray(mask).astype(np.uint8))
    W_dec = np.ascontiguousarray(np.asarray(W_dec, dtype=np.float32))
    W_enc = np.ascontiguousarray(np.asarray(W_enc, dtype=np.float32))
    v = np.ascontiguousarray(np.asarray(v, dtype=np.float32))

    nc = _get_nc()
    in_maps = []
    for c in range(NCORES):
        sl = slice(c * BPC, (c + 1) * BPC)
        in_maps.append({
            "decoder_hidden_t": np.ascontiguousarray(decoder_hidden[sl].T),
            "encoder_outputs": encoder_outputs[sl],
            "mask_t": np.ascontiguousarray(
                mask_u8[sl].reshape(BPC, NCH, SC).transpose(0, 2, 1)),
            "W_dec": W_dec,
            "W_enc": W_enc,
            "v": v,
        })
    res = run_bass_kernel_spmd(nc, in_maps, list(range(NCORES)),
                               trace=trace, **run_kwargs)
    context = np.concatenate([res.results[c]["context_out"] for c in range(NCORES)])
    attn = np.concatenate([res.results[c]["attn_out"] for c in range(NCORES)])
    if trace:
        kernel.last_results = res
    return context, attn


# revision 19
# speedup vs baseline: 1.2368x; 1.2368x over previous
"""Bahdanau attention TRN2 Bass kernel.

Data-parallel over batch across 8 NeuronCores (8 batch rows per core).
Single pass over encoder_outputs per core (fused scores + softmax + context):

  per (batch b, pair of S-chunks of 128 rows):
    enc_sb [128(S), 2, 1024(E)] <- SWDGE cast-DMA fp32->bf16 (natural layout)
    per chunk: encT [E, S] via 8x PE transpose (bf16) -> PSUM -> SBUF copy
               enc_proj [S, A] = encT.T @ W_enc (8 K-steps bf16, PSUM accum)
    z = enc_proj + dec_proj[b] (DVE); tanh (ACT)
    scores[S,2] = reduce_add(tanh * v) + mask_bias (DVE)
    w = exp(scores) (ACT; masked rows carry -1e10 bias -> exp == 0)
    ctx[1,1024] += w.T @ enc_sb  (bf16, PSUM accumulate across chunks)
  batch end: denom via ones-matmul (fused broadcast to 32 partitions),
  normalize ctx and attention weights.

Scores are bounded (|score| <= sum|v| ~ 10), so no running-max is needed and
exp/softmax matches the reference (which subtracts the max) to fp32 rounding.
decoder_hidden and mask are fed in DMA-friendly layouts (host-side transpose
only; all arithmetic stays on device).
"""

import sys

sys.path.insert(0, "/opt/trn_rl_repo")

import numpy as np
from contextlib import ExitStack

import concourse.bass as bass
import concourse.tile as tile
from concourse import bacc, mybir
from concourse.bass import ds, ts
from concourse.bass_utils import run_bass_kernel_spmd
from concourse.masks import make_identity

F32 = mybir.dt.float32
BF16 = mybir.dt.bfloat16
U8 = mybir.dt.uint8

# problem shape (hardcoded per harness contract)
B, S, DEC, ENC, A = 64, 4096, 512, 1024, 256
NCORES = 8
BPC = B // NCORES  # batches per core
SC = 128           # S rows per chunk
NCH = S // SC      # 32 chunks

COPY_ACT_K = 5     # of 8 encT k-slices copied on ACT (rest on DVE)


def build_program(n_batches=BPC, n_chunks=NCH, num_devices=NCORES,
                  enc_bufs=6, copy_act_k=COPY_ACT_K,
                  pst_bufs=4, psep_bufs=2, psctx_bufs=1, enct_bufs=3,
                  work_bufs=3):
    nc = bacc.Bacc("TRN2", target_bir_lowering=False, debug=False,
                   num_devices=num_devices)

    dh_d = nc.dram_tensor("decoder_hidden_t", [DEC, BPC], F32, kind="ExternalInput")
    enc_d = nc.dram_tensor("encoder_outputs", [BPC, S, ENC], F32, kind="ExternalInput")
    mask_d = nc.dram_tensor("mask_t", [BPC, SC, NCH], U8, kind="ExternalInput")
    wdec_d = nc.dram_tensor("W_dec", [DEC, A], F32, kind="ExternalInput")
    wenc_d = nc.dram_tensor("W_enc", [ENC, A], F32, kind="ExternalInput")
    v_d = nc.dram_tensor("v", [A], F32, kind="ExternalInput")
    ctx_d = nc.dram_tensor("context_out", [BPC, ENC], F32, kind="ExternalOutput")
    attn_d = nc.dram_tensor("attn_out", [BPC, S], F32, kind="ExternalOutput")

    KE = ENC // 128  # 8 contraction chunks for enc proj
    KD = DEC // 128  # 4 contraction chunks for dec proj
    npairs = n_chunks // 2

    with ExitStack() as ctx:
        tc = ctx.enter_context(tile.TileContext(nc))
        singles = ctx.enter_context(tc.tile_pool(name="singles", bufs=1))
        encp = ctx.enter_context(tc.tile_pool(name="encp", bufs=enc_bufs))
        enctp = ctx.enter_context(tc.tile_pool(name="enctp", bufs=enct_bufs))
        work = ctx.enter_context(tc.tile_pool(name="work", bufs=work_bufs))
        perb = ctx.enter_context(tc.tile_pool(name="perb", bufs=2))
        outp = ctx.enter_context(tc.tile_pool(name="outp", bufs=2))
        ps_t = ctx.enter_context(tc.tile_pool(name="ps_t", bufs=pst_bufs, space="PSUM"))
        ps_ep = ctx.enter_context(tc.tile_pool(name="ps_ep", bufs=psep_bufs, space="PSUM"))
        ps_ctx = ctx.enter_context(tc.tile_pool(name="ps_ctx", bufs=psctx_bufs, space="PSUM"))
        dramp = ctx.enter_context(tc.tile_pool(name="dramp", bufs=1, space="DRAM"))

        # ---- constants / prologue ----
        ident = singles.tile([128, 128], F32)
        make_identity(nc, ident[:])
        ident_b = singles.tile([128, 128], BF16)
        nc.vector.tensor_copy(ident_b[:], ident[:])

        wenc_sb = singles.tile([128, KE, A], BF16)
        nc.gpsimd.dma_start(out=wenc_sb[:],
                            in_=wenc_d.ap().rearrange("(k p) a -> p k a", p=128))
        wdec_sb = singles.tile([128, KD, A], F32)
        nc.sync.dma_start(out=wdec_sb[:],
                          in_=wdec_d.ap().rearrange("(k p) a -> p k a", p=128))
        v_bc = singles.tile([128, 1, A], F32)
        nc.sync.dma_start(out=v_bc[:],
                          in_=v_d.ap()[None, None, :].to_broadcast((128, 1, A)))
        ones32 = singles.tile([128, NCH], F32)
        nc.vector.memset(ones32[:], 1.0)

        dhT = singles.tile([128, KD, BPC], F32)
        nc.sync.dma_start(out=dhT[:],
                          in_=dh_d.ap().rearrange("(k p) b -> p k b", p=128))
        # dec_proj [BPC, A]
        dp_ps = ps_ep.tile([128, 2, A], F32, tag="ps_ep")
        for k in range(KD):
            nc.tensor.matmul(dp_ps[:BPC, 0, :], dhT[:, k, :], wdec_sb[:, k, :],
                             start=(k == 0), stop=(k == KD - 1))
        dp_sb = singles.tile([BPC, A], F32)
        nc.vector.tensor_copy(dp_sb[:], dp_ps[:BPC, 0, :])
        dp_dram = dramp.tile([BPC, A], F32)
        nc.sync.dma_start(out=dp_dram[:], in_=dp_sb[:])
        dec_bc_all = singles.tile([128, BPC, A], F32)
        nc.sync.dma_start(out=dec_bc_all[:],
                          in_=dp_dram[None, :, :].to_broadcast((128, BPC, A)))

        mask_all = singles.tile([SC, BPC, NCH], U8)
        nc.sync.dma_start(out=mask_all[:],
                          in_=mask_d.ap().rearrange("b p c -> p b c"))
        mbias_all = singles.tile([128, BPC, NCH], F32)
        nc.scalar.activation(mbias_all[:], mask_all[:],
                             mybir.ActivationFunctionType.Copy,
                             bias=0.0, scale=-1e10)

        for b in range(n_batches):
            dec_bc = dec_bc_all[:, b:b + 1, :]
            mbiasT = mbias_all[:, b, :]

            w_all = perb.tile([128, NCH], F32)
            ctx_ps = [ps_ctx.tile([1, 512], F32, tag=f"ps_ctx{i}", name=f"ctx_ps{i}_{b}")
                      for i in range(2)]

            for q in range(npairs):
                enc_sb = encp.tile([128, 2, ENC], BF16)
                nc.gpsimd.dma_start(
                    out=enc_sb[:],
                    in_=enc_d[b, ds(q * 2 * SC, 2 * SC), :]
                        .rearrange("(j p) e -> p j e", p=SC))

                ep_ps = ps_ep.tile([128, 2, A], F32, tag="ps_ep")
                for j in range(2):
                    encT_ps = ps_t.tile([128, KE, 128], BF16)
                    for k in range(KE):
                        nc.tensor.transpose(encT_ps[:, k, :],
                                            enc_sb[:, j, ts(k, 128)], ident_b[:])
                    encT = enctp.tile([128, KE, 128], BF16)
                    ka = copy_act_k
                    nc.scalar.copy(encT[:, :ka, :], encT_ps[:, :ka, :])
                    nc.vector.tensor_copy(encT[:, ka:, :], encT_ps[:, ka:, :])
                    for k in range(KE):
                        nc.tensor.matmul(ep_ps[:, j, :], encT[:, k, :],
                                         wenc_sb[:, k, :],
                                         start=(k == 0), stop=(k == KE - 1))

                # z = enc_proj + dec_proj[b]; tanh; scores = v . tanh + mask
                z_sb = work.tile([128, 2, A], F32)
                nc.vector.tensor_add(z_sb[:], ep_ps[:],
                                     dec_bc.to_broadcast((128, 2, A)))
                th_sb = work.tile([128, 2, A], F32)
                nc.scalar.activation(th_sb[:], z_sb[:],
                                     mybir.ActivationFunctionType.Tanh)
                prod_sb = work.tile([128, 2, A], F32)
                nc.vector.tensor_mul(prod_sb[:], th_sb[:],
                                     v_bc[:].to_broadcast((128, 2, A)))
                sc_sb = work.tile([128, 2], F32)
                nc.vector.reduce_sum(sc_sb[:], prod_sb[:],
                                     axis=mybir.AxisListType.X)
                scm_sb = work.tile([128, 2], F32)
                nc.vector.tensor_add(scm_sb[:], sc_sb[:],
                                     mbiasT[:, 2 * q:2 * q + 2])
                # w = exp(scores); masked rows underflow to exactly 0
                nc.scalar.activation(w_all[:, 2 * q:2 * q + 2], scm_sb[:],
                                     mybir.ActivationFunctionType.Exp)
                w_bf = work.tile([128, 2], BF16)
                nc.vector.tensor_copy(w_bf[:], w_all[:, 2 * q:2 * q + 2])

                # ctx += w.T @ enc
                for j in range(2):
                    c = 2 * q + j
                    for i in range(2):
                        nc.tensor.matmul(ctx_ps[i][:], w_bf[:, j:j + 1],
                                         enc_sb[:, j, ts(i, 512)],
                                         start=(c == 0), stop=(c == n_chunks - 1),
                                         skip_group_check=True)

            # ---- batch epilogue ----
            dsum = outp.tile([128, 1], F32)
            nc.vector.reduce_sum(dsum[:], w_all[:, :n_chunks],
                                 axis=mybir.AxisListType.X)
            den_ps = ps_ep.tile([128, 2, A], F32, tag="ps_ep")
            nc.tensor.matmul(den_ps[:NCH, 0, :1], ones32[:], dsum[:],
                             start=True, stop=True)
            den32 = outp.tile([NCH, 1], F32)
            nc.vector.tensor_copy(den32[:], den_ps[:NCH, 0, :1])
            rden32 = outp.tile([NCH, 1], F32)
            nc.vector.reciprocal(rden32[:], den32[:])

            # context out
            ctx_sb = outp.tile([1, ENC], F32)
            for i in range(2):
                nc.vector.tensor_scalar_mul(ctx_sb[:, ts(i, 512)], ctx_ps[i][:],
                                            rden32[:1, :])
            nc.sync.dma_start(out=ctx_d[b][None, :], in_=ctx_sb[:])

            # attention weights out: transpose w_all -> [chunk, S-in-chunk]
            wT_ps = ps_ep.tile([128, 2, A], F32, tag="ps_ep")
            nc.tensor.transpose(wT_ps[:n_chunks, 0, :SC], w_all[:, :n_chunks],
                                ident[:])
            wout = outp.tile([NCH, SC], F32)
            nc.vector.tensor_scalar_mul(wout[:n_chunks], wT_ps[:n_chunks, 0, :SC],
                                        rden32[:n_chunks])
            nc.sync.dma_start(
                out=attn_d[b].rearrange("(c p) -> c p", p=SC)[:n_chunks],
                in_=wout[:n_chunks])

    nc.compile()
    return nc


_NC = None


def _get_nc():
    global _NC
    if _NC is None:
        _NC = build_program()
    return _NC


def kernel(decoder_hidden, encoder_outputs, mask, W_dec, W_enc, v,
           trace=False, **run_kwargs):
    decoder_hidden = np.ascontiguousarray(np.asarray(decoder_hidden, dtype=np.float32))
    encoder_outputs = np.ascontiguousarray(np.asarray(encoder_outputs, dtype=np.float32))
    mask_u8 = np.ascontiguousarray(np.asarray(mask).astype(np.uint8))
    W_dec = np.ascontiguousarray(np.asarray(W_dec, dtype=np.float32))
    W_enc = np.ascontiguousarray(np.asarray(W_enc, dtype=np.float32))
    v = np.ascontiguousarray(np.asarray(v, dtype=np.float32))

    nc = _get_nc()
    in_maps = []
    for c in range(NCORES):
        sl = slice(c * BPC, (c + 1) * BPC)
        in_maps.append({
            "decoder_hidden_t": np.ascontiguousarray(decoder_hidden[sl].T),
            "encoder_outputs": encoder_outputs[sl],
            "mask_t": np.ascontiguousarray(
                mask_u8[sl].reshape(BPC, NCH, SC).transpose(0, 2, 1)),
            "W_dec": W_dec,
            "W_enc": W_enc,
            "v": v,
        })
    res = run_bass_kernel_spmd(nc, in_maps, list(range(NCORES)),
                               trace=trace, **run_kwargs)
    context = np.concatenate([res.results[c]["context_out"] for c in range(NCORES)])
    attn = np.concatenate([res.results[c]["attn_out"] for c in range(NCORES)])
    if trace:
        kernel.last_results = res
    return context, attn
